# revision 8
# baseline (speedup 1.0000x reference)
"""Trainium2 Bass kernel for nn_Block_7627861918171 (dense transformer block).

Sharding: 8 cores = 2 batches x 4 query-chunks of 560 tokens. Each core
receives its batch's x (and the positional table) *rolled* so that its
query window is always tokens [0, 560) -- one fixed SPMD program for all
cores. The attention-probability output comes back with rolled key
columns and is un-rolled on the host during the gather.

Per core the device program computes, in fp32 (matmuls via the PE's
fp32r fast path):
  x+pos -> LN1 (gain/bias folded into the QKV weights on the host) ->
  PE-transpose to h^T -> K^T / V / Q^T -> per head: scores in both
  orientations ([q,k] for the attn output + row sums via ACT accum_out;
  [k,q] for attnU @ V), exp straight out of PSUM on ACT, normalize on
  DVE -> proj (bias via rank-1 matmul) -> residual -> LN2 -> FC1 +
  exact Gelu -> FC2 -> residual -> y.
"""

import os
import numpy as np

N = 2240
C = 384
H = 6
HD = 64
FF = 1536
NCORES = 8
NLOC = N // 4          # 560 queries per core
EPS = 1e-5
P = 128

USE_F32R = os.environ.get("KERNEL_NO_F32R", "") == ""

TOKB = [(i * P, min(P, N - i * P)) for i in range((N + P - 1) // P)]   # 18 blocks
QB = [(0, 128), (128, 128), (256, 128), (384, 128), (512, 48)]          # 560 rows
KW = 448                                                                # o1 key chunk
QCH = [(0, 280), (280, 280)]                                            # o2 q chunks

_PROG = None
LAST_RESULT = None


def build_program():
    from contextlib import ExitStack

    import concourse.bass as bass  # noqa: F401
    from concourse import bacc, mybir
    from concourse.masks import make_identity
    from concourse.tile import TileContext

    f32 = mybir.dt.float32
    f32r = mybir.dt.float32r
    AF = mybir.ActivationFunctionType
    ALU = mybir.AluOpType
    AX = mybir.AxisListType

    MMDT = f32r if USE_F32R else f32

    def mm(ap):
        return ap

    nc = bacc.Bacc("TRN2", target_bir_lowering=False, debug=False,
                   num_devices=NCORES)

    dx = nc.dram_tensor("x", [N, C], f32, kind="ExternalInput").ap()
    dpos = nc.dram_tensor("pos", [N, C], f32, kind="ExternalInput").ap()
    dwq = nc.dram_tensor("wqT", [C, C], MMDT, kind="ExternalInput").ap()
    dwk = nc.dram_tensor("wkT", [C, C], MMDT, kind="ExternalInput").ap()
    dwv = nc.dram_tensor("wvT", [C, C], MMDT, kind="ExternalInput").ap()
    dwp = nc.dram_tensor("wpT", [C, C], MMDT, kind="ExternalInput").ap()
    dwf1 = nc.dram_tensor("wf1T", [C, FF], MMDT, kind="ExternalInput").ap()
    dwf2 = nc.dram_tensor("wf2T", [FF, C], MMDT, kind="ExternalInput").ap()
    dbq = nc.dram_tensor("bq", [P, 3], f32, kind="ExternalInput").ap()
    dbf1 = nc.dram_tensor("bf1", [P, 12], f32, kind="ExternalInput").ap()
    dbp = nc.dram_tensor("bp", [1, C], MMDT, kind="ExternalInput").ap()
    dbf2 = nc.dram_tensor("bf2", [1, C], MMDT, kind="ExternalInput").ap()

    dattn = nc.dram_tensor("attn_out", [H, NLOC, N], f32,
                           kind="ExternalOutput").ap()
    dy = nc.dram_tensor("y_out", [NLOC, C], f32, kind="ExternalOutput").ap()
    drs = nc.dram_tensor("rs_scratch", [H, NLOC], f32).ap()

    with TileContext(nc) as tc, ExitStack() as ctx:
        persist = ctx.enter_context(tc.tile_pool(name="persist", bufs=1))

        WQ = persist.tile([P, 3, C], MMDT, tag="WQ")
        WK = persist.tile([P, 3, C], MMDT, tag="WK")
        WV = persist.tile([P, 3, C], MMDT, tag="WV")
        WP = persist.tile([P, 3, C], MMDT, tag="WP")
        WF1 = persist.tile([P, 3, FF], MMDT, tag="WF1")
        WF2 = persist.tile([P, 12, C], MMDT, tag="WF2")
        BQ = persist.tile([P, 3], f32, tag="BQ")
        BF1 = persist.tile([P, 12], f32, tag="BF1")
        BP = persist.tile([1, C], MMDT, tag="BP")
        BF2 = persist.tile([1, C], MMDT, tag="BF2")
        IDT = persist.tile([P, P], f32, tag="IDT")
        ONES = persist.tile([1, NLOC], MMDT, tag="ONES")
        EPSB = persist.tile([P, 1], f32, tag="EPSB")

        KT = persist.tile([P, 3, N], MMDT, tag="KT")
        Vt = persist.tile([P, len(TOKB), C], MMDT, tag="Vt")
        QT = persist.tile([P, 3, NLOC], MMDT, tag="QT")
        XPL = persist.tile([P, len(QB), C], f32, tag="XPL")
        OT = persist.tile([P, 3, NLOC], MMDT, tag="OT")

        nc.sync.dma_start(out=WQ[:], in_=dwq.rearrange("(k p) n -> p k n", p=P))
        nc.sync.dma_start(out=WK[:], in_=dwk.rearrange("(k p) n -> p k n", p=P))
        nc.sync.dma_start(out=WV[:], in_=dwv.rearrange("(k p) n -> p k n", p=P))
        nc.sync.dma_start(out=WP[:], in_=dwp.rearrange("(k p) n -> p k n", p=P))
        nc.sync.dma_start(out=WF1[:], in_=dwf1.rearrange("(k p) n -> p k n", p=P))
        nc.sync.dma_start(out=WF2[:], in_=dwf2.rearrange("(k p) n -> p k n", p=P))
        nc.sync.dma_start(out=BQ[:], in_=dbq)
        nc.sync.dma_start(out=BF1[:], in_=dbf1)
        nc.sync.dma_start(out=BP[:], in_=dbp)
        nc.sync.dma_start(out=BF2[:], in_=dbf2)
        make_identity(nc, IDT[:])
        nc.vector.memset(EPSB[:], EPS)
        ONESF = persist.tile([1, NLOC], f32, tag="ONESF")
        nc.vector.memset(ONESF[:], 1.0)
        nc.vector.tensor_copy(ONES[:], ONESF[:])

        def layernorm(lnp, src_ap, dst_ap, tp):
            """dst = (src - mean) * rsqrt(var + eps) for [tp, C] tiles."""
            st = lnp.tile([P, 6], f32, tag="st")
            mv = lnp.tile([P, 2], f32, tag="mv")
            nc.vector.bn_stats(out=st[:tp], in_=src_ap)
            nc.vector.bn_aggr(out=mv[:tp], in_=st[:tp])
            nc.scalar.activation(out=mv[:tp, 1:2], in_=mv[:tp, 1:2],
                                 func=AF.Sqrt, bias=EPSB[:tp, :])
            nc.vector.reciprocal(out=mv[:tp, 1:2], in_=mv[:tp, 1:2])
            nc.vector.tensor_scalar(out=dst_ap, in0=src_ap,
                                    scalar1=mv[:tp, 0:1], scalar2=mv[:tp, 1:2],
                                    op0=ALU.subtract, op1=ALU.mult)

        # ---------------- Phase 1+2: x+pos, LN1, h^T, K^T, V, Q^T ---------
        with ExitStack() as ph:
            hT_pool = ph.enter_context(tc.tile_pool(name="hTp", bufs=1))
            io = ph.enter_context(tc.tile_pool(name="io", bufs=3))
            lnp = ph.enter_context(tc.tile_pool(name="ln1", bufs=4))
            tpp = ph.enter_context(tc.tile_pool(name="tpsum", bufs=2, space="PSUM"))
            ktp = ph.enter_context(tc.tile_pool(name="ktpsum", bufs=2, space="PSUM"))
            vpp = ph.enter_context(tc.tile_pool(name="vpsum", bufs=2, space="PSUM"))
            qtp = ph.enter_context(tc.tile_pool(name="qtpsum", bufs=2, space="PSUM"))

            hT = hT_pool.tile([P, 3, N], MMDT, tag="hT")

            for bt, (t0, tp) in enumerate(TOKB):
                xt = io.tile([P, C], f32, tag="xt")
                pt = io.tile([P, C], f32, tag="pt")
                nc.sync.dma_start(out=xt[:tp], in_=dx[t0:t0 + tp, :])
                nc.sync.dma_start(out=pt[:tp], in_=dpos[t0:t0 + tp, :])
                if bt < len(QB):
                    xp = XPL[0:tp, bt, :]
                else:
                    xps = io.tile([P, C], f32, tag="xp")
                    xp = xps[0:tp, :]
                nc.vector.tensor_add(out=xp, in0=xt[:tp], in1=pt[:tp])

                hn = lnp.tile([P, C], f32, tag="hn")
                layernorm(lnp, xp, hn[:tp], tp)
                for m in range(3):
                    tps = tpp.tile([P, P], f32, tag="tp")
                    nc.tensor.transpose(out=tps[:, :tp],
                                        in_=hn[:tp, m * P:(m + 1) * P],
                                        identity=IDT[:tp, :tp])
                    nc.scalar.copy(out=hT[:, m, t0:t0 + tp], in_=tps[:, :tp])

            # K^T [C, N] in 5 chunks of 448
            for m in range(3):
                for ci in range(5):
                    n0 = ci * KW
                    kps = ktp.tile([P, KW], f32, tag="kt")
                    for kk in range(3):
                        nc.tensor.matmul(out=kps[:, :],
                                         lhsT=mm(WK[:, kk, m * P:(m + 1) * P]),
                                         rhs=mm(hT[:, kk, n0:n0 + KW]),
                                         start=(kk == 0), stop=(kk == 2))
                    nc.vector.tensor_copy(KT[:, m, n0:n0 + KW], kps[:, :])

            # V [N, C]
            for bt, (t0, tp) in enumerate(TOKB):
                vps = vpp.tile([P, C], f32, tag="v")
                for kk in range(3):
                    nc.tensor.matmul(out=vps[:tp, :],
                                     lhsT=mm(hT[:, kk, t0:t0 + tp]),
                                     rhs=mm(WV[:, kk, :]),
                                     start=(kk == 0), stop=(kk == 2))
                nc.vector.tensor_copy(Vt[0:tp, bt, :], vps[:tp, :])

            # Q^T [C, NLOC] (+ folded-LN bias, scaled by hd^-0.5 on host)
            for m in range(3):
                for (q0, qw) in QCH:
                    qps = qtp.tile([P, 280], f32, tag="qt")
                    for kk in range(3):
                        nc.tensor.matmul(out=qps[:, :qw],
                                         lhsT=mm(WQ[:, kk, m * P:(m + 1) * P]),
                                         rhs=mm(hT[:, kk, q0:q0 + qw]),
                                         start=(kk == 0), stop=(kk == 2))
                    nc.vector.tensor_scalar(out=QT[:, m, q0:q0 + qw],
                                            in0=qps[:, :qw],
                                            scalar1=BQ[:, m:m + 1], scalar2=None,
                                            op0=ALU.add)

        # ---------------- Phase 3: attention ------------------------------
        with ExitStack() as ph:
            o1p = ph.enter_context(tc.tile_pool(name="o1psum", bufs=1, space="PSUM"))
            o2p = ph.enter_context(tc.tile_pool(name="o2psum", bufs=1, space="PSUM"))
            otp = ph.enter_context(tc.tile_pool(name="otpsum", bufs=1, space="PSUM"))
            aup = ph.enter_context(tc.tile_pool(name="attnu", bufs=2))
            autp = ph.enter_context(tc.tile_pool(name="attnut", bufs=3))
            smp = ph.enter_context(tc.tile_pool(name="sums", bufs=4))
            rrp = ph.enter_context(tc.tile_pool(name="rsrow", bufs=2))

            for h in range(H):
                hm, hr = divmod(h, 2)
                QTh = QT[hr * HD:(hr + 1) * HD, hm, :]   # [64, NLOC]
                KTh = KT[hr * HD:(hr + 1) * HD, hm, :]   # [64, N]
                rsrow = rrp.tile([1, NLOC], f32, tag="rsrow")

                # -- orientation 1: attn[q, k] for the HBM write + row sums
                for qi, (q0, qw) in enumerate(QB):
                    attnu = aup.tile([P, N], f32, tag="au")
                    sums = smp.tile([P, 4], f32, tag="sums")
                    for pi, tag in ((0, "o1A"), (1, "o1B")):
                        ps = o1p.tile([P, 1024], f32, tag=tag)
                        for j in range(2):
                            n0 = (pi * 2 + j) * KW
                            nc.tensor.matmul(out=ps[:qw, j * 512:j * 512 + KW],
                                             lhsT=mm(QTh[:, q0:q0 + qw]),
                                             rhs=mm(KTh[:, n0:n0 + KW]),
                                             start=True, stop=True)
                        nc.scalar.activation(
                            out=attnu[0:qw, pi * 2 * KW:(pi * 2 + 2) * KW]
                                .rearrange("p (c n) -> p c n", c=2),
                            in_=ps[0:qw, :].rearrange("p (c n) -> p c n", c=2)[:, :, 0:KW],
                            func=AF.Exp, accum_out=sums[0:qw, pi:pi + 1])
                    ps = o1p.tile([P, 1024], f32, tag="o1A")
                    nc.tensor.matmul(out=ps[:qw, 0:KW],
                                     lhsT=mm(QTh[:, q0:q0 + qw]),
                                     rhs=mm(KTh[:, 4 * KW:5 * KW]),
                                     start=True, stop=True)
                    nc.scalar.activation(out=attnu[0:qw, 4 * KW:5 * KW],
                                         in_=ps[0:qw, 0:KW],
                                         func=AF.Exp, accum_out=sums[0:qw, 2:3])

                    rs = smp.tile([P, 1], f32, tag="rs")
                    nc.vector.reduce_sum(out=rs[:qw, :], in_=sums[0:qw, 0:3],
                                         axis=AX.X)
                    nc.vector.reciprocal(out=rs[:qw, :], in_=rs[:qw, :])
                    nc.vector.tensor_scalar(out=attnu[0:qw, :], in0=attnu[0:qw, :],
                                            scalar1=rs[:qw, :], scalar2=None,
                                            op0=ALU.mult)
                    nc.sync.dma_start(out=dattn[h, q0:q0 + qw, :],
                                      in_=attnu[0:qw, :])

                    tps = o1p.tile([P, 1024], f32, tag="o1A")
                    nc.tensor.transpose(out=tps[0:1, 0:qw], in_=rs[0:qw, 0:1],
                                        identity=IDT[:qw, :qw])
                    nc.scalar.copy(out=rsrow[0:1, q0:q0 + qw], in_=tps[0:1, 0:qw])

                # reciprocal row sums -> broadcast over 64 partitions
                rsbc = rrp.tile([HD, NLOC], f32, tag="rsbc")
                nc.sync.dma_start(out=drs[h:h + 1, :], in_=rsrow[0:1, :])
                nc.sync.dma_start(out=rsbc[:, :],
                                  in_=drs[h:h + 1, :].to_broadcast([HD, NLOC]))

                # -- orientation 2: attnU^T[k, q] and o^T = V^T @ attnU^T
                ot0 = otp.tile([HD, 280], f32, tag="ot0")
                ot1 = otp.tile([HD, 280], f32, tag="ot1")
                for bt, (t0, tp) in enumerate(TOKB):
                    o2ps = o2p.tile([P, 1024], f32, tag="o2")
                    for j, (q0, qw) in enumerate(QCH):
                        nc.tensor.matmul(out=o2ps[:tp, j * 512:j * 512 + qw],
                                         lhsT=mm(KTh[:, t0:t0 + tp]),
                                         rhs=mm(QTh[:, q0:q0 + qw]),
                                         start=True, stop=True)
                    attnut = autp.tile([P, NLOC], MMDT, tag="aut")
                    nc.scalar.activation(
                        out=attnut[0:tp, :].rearrange("p (c n) -> p c n", c=2),
                        in_=o2ps[0:tp, :].rearrange("p (c n) -> p c n", c=2)[:, :, 0:280],
                        func=AF.Exp)
                    for j, ot in ((0, ot0), (1, ot1)):
                        nc.tensor.matmul(out=ot[:, :],
                                         lhsT=mm(Vt[0:tp, bt, h * HD:(h + 1) * HD]),
                                         rhs=mm(attnut[0:tp, j * 280:(j + 1) * 280]),
                                         start=(bt == 0), stop=(bt == len(TOKB) - 1))
                for j, ot in ((0, ot0), (1, ot1)):
                    nc.vector.tensor_mul(
                        out=OT[hr * HD:(hr + 1) * HD, hm, j * 280:(j + 1) * 280],
                        in0=ot[:, :], in1=rsbc[:, j * 280:(j + 1) * 280])

        # ---------------- Phase 4: proj + residual + LN2 + h2^T -----------
        with ExitStack() as ph:
            pjp = ph.enter_context(tc.tile_pool(name="pjpsum", bufs=2, space="PSUM"))
            t2p = ph.enter_context(tc.tile_pool(name="t2psum", bufs=2, space="PSUM"))
            h2Tp = ph.enter_context(tc.tile_pool(name="h2Tp", bufs=1))
            lnp2 = ph.enter_context(tc.tile_pool(name="ln2", bufs=4))

            h2T = h2Tp.tile([P, 3, NLOC], MMDT, tag="h2T")

            for qi, (q0, qw) in enumerate(QB):
                pj = pjp.tile([P, C], f32, tag="pj")
                for kk in range(3):
                    nc.tensor.matmul(out=pj[:qw, :],
                                     lhsT=mm(OT[:, kk, q0:q0 + qw]),
                                     rhs=mm(WP[:, kk, :]),
                                     start=(kk == 0), stop=False)
                nc.tensor.matmul(out=pj[:qw, :], lhsT=mm(ONES[0:1, 0:qw]),
                                 rhs=mm(BP[0:1, :]), start=False, stop=True)
                nc.vector.tensor_add(out=XPL[0:qw, qi, :], in0=XPL[0:qw, qi, :],
                                     in1=pj[:qw, :])
                h2 = lnp2.tile([P, C], f32, tag="h2")
                layernorm(lnp2, XPL[0:qw, qi, :], h2[:qw], qw)
                for m in range(3):
                    tps = t2p.tile([P, P], f32, tag="t2")
                    nc.tensor.transpose(out=tps[:, :qw],
                                        in_=h2[:qw, m * P:(m + 1) * P],
                                        identity=IDT[:qw, :qw])
                    nc.scalar.copy(out=h2T[:, m, q0:q0 + qw], in_=tps[:, :qw])

            # ------------- Phase 5: MLP -----------------------------------
            with ExitStack() as ph2:
                f1p = ph2.enter_context(tc.tile_pool(name="f1psum", bufs=2,
                                                     space="PSUM"))
                f2p = ph2.enter_context(tc.tile_pool(name="f2psum", bufs=2,
                                                     space="PSUM"))
                gtp = ph2.enter_context(tc.tile_pool(name="gT", bufs=1))
                yp = ph2.enter_context(tc.tile_pool(name="yout", bufs=3))

                gT = gtp.tile([P, 12, NLOC], MMDT, tag="gT")
                for fm in range(12):
                    for (q0, qw) in QCH:
                        fps = f1p.tile([P, 280], f32, tag="f1")
                        for kk in range(3):
                            nc.tensor.matmul(out=fps[:, :qw],
                                             lhsT=mm(WF1[:, kk, fm * P:(fm + 1) * P]),
                                             rhs=mm(h2T[:, kk, q0:q0 + qw]),
                                             start=(kk == 0), stop=(kk == 2))
                        nc.scalar.activation(out=gT[:, fm, q0:q0 + qw],
                                             in_=fps[:, :qw], func=AF.Gelu,
                                             bias=BF1[:, fm:fm + 1])

                for qi, (q0, qw) in enumerate(QB):
                    f2 = f2p.tile([P, C], f32, tag="f2")
                    for kk in range(12):
                        nc.tensor.matmul(out=f2[:qw, :],
                                         lhsT=mm(gT[:, kk, q0:q0 + qw]),
                                         rhs=mm(WF2[:, kk, :]),
                                         start=(kk == 0), stop=False)
                    nc.tensor.matmul(out=f2[:qw, :], lhsT=mm(ONES[0:1, 0:qw]),
                                     rhs=mm(BF2[0:1, :]), start=False, stop=True)
                    yt = yp.tile([P, C], f32, tag="y")
                    nc.vector.tensor_add(out=yt[:qw, :], in0=XPL[0:qw, qi, :],
                                         in1=f2[:qw, :])
                    nc.sync.dma_start(out=dy[q0:q0 + qw, :], in_=yt[:qw, :])

    nc.finalize()
    return nc


def _get_program():
    global _PROG
    if _PROG is None:
        _PROG = build_program()
    return _PROG


def tf32_round(a):
    """Round fp32 -> tf32 (10-bit mantissa, RNE). fp32r == tf32 on trn2."""
    if not USE_F32R:
        return np.asarray(a, np.float32)
    u = np.asarray(a, np.float32).view(np.uint32).astype(np.uint64)
    u = (u + 0x0FFF + ((u >> 13) & 1)) & np.uint64(0xFFFFE000)
    return u.astype(np.uint32).view(np.float32)


def host_weights(inputs):
    """Fold LN gains/biases into the weights; pre-transpose for the PE."""
    f64 = np.float64
    w_qkv = np.asarray(inputs["w_qkv"], f64)
    ln1_g = np.asarray(inputs["ln1_g"], f64)
    ln1_b = np.asarray(inputs["ln1_b"], f64)
    ln2_g = np.asarray(inputs["ln2_g"], f64)
    ln2_b = np.asarray(inputs["ln2_b"], f64)
    w_proj = np.asarray(inputs["w_proj"], f64)
    b_proj = np.asarray(inputs["b_proj"], f64)
    w_fc1 = np.asarray(inputs["w_fc1"], f64)
    b_fc1 = np.asarray(inputs["b_fc1"], f64)
    w_fc2 = np.asarray(inputs["w_fc2"], f64)
    b_fc2 = np.asarray(inputs["b_fc2"], f64)

    scale = HD ** -0.5
    wq = w_qkv[0:C] * ln1_g[None, :] * scale
    bq = (w_qkv[0:C] @ ln1_b) * scale
    wk = w_qkv[C:2 * C] * ln1_g[None, :]
    # k-bias shifts every logit in a softmax row by a constant -> no-op.
    wv = w_qkv[2 * C:3 * C] * ln1_g[None, :]
    bv = w_qkv[2 * C:3 * C] @ ln1_b
    # v-bias adds a constant vector to every o row (attn rows sum to 1):
    # fold it through the projection into the projection bias.
    bp = b_proj + w_proj @ bv
    wf1 = w_fc1 * ln2_g[None, :]
    bf1 = b_fc1 + w_fc1 @ ln2_b

    def c(a):
        return np.ascontiguousarray(a)

    def cr(a):
        return np.ascontiguousarray(tf32_round(a.astype(np.float32)))

    return {
        "wqT": cr(wq.T),
        "wkT": cr(wk.T),
        "wvT": cr(wv.T),
        "wpT": cr(w_proj.T),
        "wf1T": cr(wf1.T),
        "wf2T": cr(w_fc2.T),
        "bq": c(bq.astype(np.float32).reshape(3, P).T),
        "bf1": c(bf1.astype(np.float32).reshape(12, P).T),
        "bp": cr(bp.reshape(1, C)),
        "bf2": cr(np.asarray(b_fc2, np.float64).reshape(1, C)),
    }


def kernel(**inputs):
    global LAST_RESULT
    from concourse import bass_utils

    x = np.asarray(inputs["x"], np.float32)                 # [2, N, C]
    pos = np.asarray(inputs["encoder_pos"], np.float32)
    rep = N // pos.shape[1]
    if rep != 1:
        pos = np.repeat(pos, rep, axis=1)
    pos = pos[0]                                            # [N, C]
    base = host_weights(inputs)

    in_maps = []
    for core in range(NCORES):
        b = core // 4
        qs = (core % 4) * NLOC
        in_maps.append({
            **base,
            "x": np.ascontiguousarray(np.roll(x[b], -qs, axis=0)),
            "pos": np.ascontiguousarray(np.roll(pos, -qs, axis=0)),
        })

    nc = _get_program()
    kw = {}
    tdir = os.environ.get("BASS_KERNEL_TRACE_DIR")
    if tdir:
        kw["tmpdir"] = tdir
    res = bass_utils.run_bass_kernel_spmd(nc, in_maps,
                                          core_ids=list(range(NCORES)), **kw)
    LAST_RESULT = res

    attn = np.empty((2, H, N, N), np.float32)
    y = np.empty((2, N, C), np.float32)
    for core in range(NCORES):
        b = core // 4
        qs = (core % 4) * NLOC
        attn[b, :, qs:qs + NLOC, :] = np.roll(res.results[core]["attn_out"],
                                              qs, axis=-1)
        y[b, qs:qs + NLOC, :] = res.results[core]["y_out"]
    return (y, attn)


# revision 11
# speedup vs baseline: 1.0929x; 1.0929x over previous
"""Trainium2 Bass kernel for nn_Block_7627861918171 (dense transformer block).

Sharding: 8 cores = 2 batches x 4 query-chunks of 560 tokens. Each core
receives its batch's x (and the positional table) *rolled* so that its
query window is always tokens [0, 560) -- one fixed SPMD program for all
cores. The attention-probability output comes back with rolled key
columns and is un-rolled on the host during the gather.

Per core the device program computes, in fp32 (matmuls via the PE's
fp32r fast path):
  x+pos -> LN1 (gain/bias folded into the QKV weights on the host) ->
  PE-transpose to h^T -> K^T / V / Q^T -> per head: scores in both
  orientations ([q,k] for the attn output + row sums via ACT accum_out;
  [k,q] for attnU @ V), exp straight out of PSUM on ACT, normalize on
  DVE -> proj (bias via rank-1 matmul) -> residual -> LN2 -> FC1 +
  exact Gelu -> FC2 -> residual -> y.
"""

import os
import numpy as np

N = 2240
C = 384
H = 6
HD = 64
FF = 1536
NCORES = 8
NLOC = N // 4          # 560 queries per core
EPS = 1e-5
P = 128

USE_F32R = os.environ.get("KERNEL_NO_F32R", "") == ""

TOKB = [(i * P, min(P, N - i * P)) for i in range((N + P - 1) // P)]   # 18 blocks
QB = [(0, 128), (128, 128), (256, 128), (384, 128), (512, 48)]          # 560 rows
KW = 448                                                                # o1 key chunk
QCH = [(0, 280), (280, 280)]                                            # o2 q chunks

_PROG = None
LAST_RESULT = None


def build_program():
    from contextlib import ExitStack

    import concourse.bass as bass  # noqa: F401
    from concourse import bacc, mybir
    from concourse.masks import make_identity
    from concourse.tile import TileContext

    f32 = mybir.dt.float32
    f32r = mybir.dt.float32r
    bf16 = mybir.dt.bfloat16
    AF = mybir.ActivationFunctionType
    ALU = mybir.AluOpType
    AX = mybir.AxisListType

    MMDT = f32r if USE_F32R else f32

    def mm(ap):
        return ap

    nc = bacc.Bacc("TRN2", target_bir_lowering=False, debug=False,
                   num_devices=NCORES)

    dx = nc.dram_tensor("x", [N, C], f32, kind="ExternalInput").ap()
    dpos = nc.dram_tensor("pos", [N, C], f32, kind="ExternalInput").ap()
    dwq = nc.dram_tensor("wqT", [C, C], MMDT, kind="ExternalInput").ap()
    dwk = nc.dram_tensor("wkT", [C, C], MMDT, kind="ExternalInput").ap()
    dwv = nc.dram_tensor("wvT", [C, C], MMDT, kind="ExternalInput").ap()
    dwp = nc.dram_tensor("wpT", [C, C], MMDT, kind="ExternalInput").ap()
    dwf1 = nc.dram_tensor("wf1T", [C, FF], MMDT, kind="ExternalInput").ap()
    dwf2 = nc.dram_tensor("wf2T", [FF, C], MMDT, kind="ExternalInput").ap()
    dbq = nc.dram_tensor("bq", [P, 3], f32, kind="ExternalInput").ap()
    dbf1 = nc.dram_tensor("bf1", [P, 12], f32, kind="ExternalInput").ap()
    dbp = nc.dram_tensor("bp", [1, C], MMDT, kind="ExternalInput").ap()
    dbf2 = nc.dram_tensor("bf2", [1, C], MMDT, kind="ExternalInput").ap()

    dattn = nc.dram_tensor("attn_out", [H, NLOC, N], f32,
                           kind="ExternalOutput").ap()
    dy = nc.dram_tensor("y_out", [NLOC, C], f32, kind="ExternalOutput").ap()
    drs = nc.dram_tensor("rs_scratch", [H, NLOC], f32).ap()

    with TileContext(nc) as tc, ExitStack() as ctx:
        persist = ctx.enter_context(tc.tile_pool(name="persist", bufs=1))

        WQ = persist.tile([P, 3, C], MMDT, tag="WQ")
        WK = persist.tile([P, 3, C], MMDT, tag="WK")
        WV = persist.tile([P, 3, C], MMDT, tag="WV")
        WP = persist.tile([P, 3, C], MMDT, tag="WP")
        WF1 = persist.tile([P, 3, FF], MMDT, tag="WF1")
        WF2 = persist.tile([P, 12, C], MMDT, tag="WF2")
        BQ = persist.tile([P, 3], f32, tag="BQ")
        BF1 = persist.tile([P, 12], f32, tag="BF1")
        BP = persist.tile([1, C], MMDT, tag="BP")
        BF2 = persist.tile([1, C], MMDT, tag="BF2")
        IDT = persist.tile([P, P], f32, tag="IDT")
        ONES = persist.tile([1, NLOC], MMDT, tag="ONES")
        EPSB = persist.tile([P, 1], f32, tag="EPSB")

        KT = persist.tile([P, 3, N], MMDT, tag="KT")
        KTB = persist.tile([P, 3, N], bf16, tag="KTB")
        Vt = persist.tile([P, len(TOKB), C], bf16, tag="Vt")
        QT = persist.tile([P, 3, NLOC], MMDT, tag="QT")
        QTB = persist.tile([P, 3, NLOC], bf16, tag="QTB")
        XPL = persist.tile([P, len(QB), C], f32, tag="XPL")
        OT = persist.tile([P, 3, NLOC], MMDT, tag="OT")

        nc.sync.dma_start(out=WQ[:], in_=dwq.rearrange("(k p) n -> p k n", p=P))
        nc.sync.dma_start(out=WK[:], in_=dwk.rearrange("(k p) n -> p k n", p=P))
        nc.sync.dma_start(out=WV[:], in_=dwv.rearrange("(k p) n -> p k n", p=P))
        nc.sync.dma_start(out=WP[:], in_=dwp.rearrange("(k p) n -> p k n", p=P))
        nc.sync.dma_start(out=WF1[:], in_=dwf1.rearrange("(k p) n -> p k n", p=P))
        nc.sync.dma_start(out=WF2[:], in_=dwf2.rearrange("(k p) n -> p k n", p=P))
        nc.sync.dma_start(out=BQ[:], in_=dbq)
        nc.sync.dma_start(out=BF1[:], in_=dbf1)
        nc.sync.dma_start(out=BP[:], in_=dbp)
        nc.sync.dma_start(out=BF2[:], in_=dbf2)
        make_identity(nc, IDT[:])
        nc.vector.memset(EPSB[:], EPS)
        ONESF = persist.tile([1, NLOC], f32, tag="ONESF")
        nc.vector.memset(ONESF[:], 1.0)
        nc.vector.tensor_copy(ONES[:], ONESF[:])

        def layernorm(lnp, src_ap, dst_ap, tp):
            """dst = (src - mean) * rsqrt(var + eps) for [tp, C] tiles."""
            st = lnp.tile([P, 6], f32, tag="st")
            mv = lnp.tile([P, 2], f32, tag="mv")
            nc.vector.bn_stats(out=st[:tp], in_=src_ap)
            nc.vector.bn_aggr(out=mv[:tp], in_=st[:tp])
            nc.scalar.activation(out=mv[:tp, 1:2], in_=mv[:tp, 1:2],
                                 func=AF.Sqrt, bias=EPSB[:tp, :])
            nc.vector.reciprocal(out=mv[:tp, 1:2], in_=mv[:tp, 1:2])
            nc.vector.tensor_scalar(out=dst_ap, in0=src_ap,
                                    scalar1=mv[:tp, 0:1], scalar2=mv[:tp, 1:2],
                                    op0=ALU.subtract, op1=ALU.mult)

        # ---------------- Phase 1+2: x+pos, LN1, h^T, K^T, V, Q^T ---------
        with ExitStack() as ph:
            hT_pool = ph.enter_context(tc.tile_pool(name="hTp", bufs=1))
            io = ph.enter_context(tc.tile_pool(name="io", bufs=3))
            lnp = ph.enter_context(tc.tile_pool(name="ln1", bufs=4))
            tpp = ph.enter_context(tc.tile_pool(name="tpsum", bufs=2, space="PSUM"))
            ktp = ph.enter_context(tc.tile_pool(name="ktpsum", bufs=2, space="PSUM"))
            vpp = ph.enter_context(tc.tile_pool(name="vpsum", bufs=2, space="PSUM"))
            qtp = ph.enter_context(tc.tile_pool(name="qtpsum", bufs=2, space="PSUM"))

            hT = hT_pool.tile([P, 3, N], MMDT, tag="hT")

            for bt, (t0, tp) in enumerate(TOKB):
                xt = io.tile([P, C], f32, tag="xt")
                pt = io.tile([P, C], f32, tag="pt")
                nc.sync.dma_start(out=xt[:tp], in_=dx[t0:t0 + tp, :])
                nc.sync.dma_start(out=pt[:tp], in_=dpos[t0:t0 + tp, :])
                if bt < len(QB):
                    xp = XPL[0:tp, bt, :]
                else:
                    xps = io.tile([P, C], f32, tag="xp")
                    xp = xps[0:tp, :]
                nc.vector.tensor_add(out=xp, in0=xt[:tp], in1=pt[:tp])

                hn = lnp.tile([P, C], f32, tag="hn")
                layernorm(lnp, xp, hn[:tp], tp)
                for m in range(3):
                    tps = tpp.tile([P, P], f32, tag="tp")
                    nc.tensor.transpose(out=tps[:, :tp],
                                        in_=hn[:tp, m * P:(m + 1) * P],
                                        identity=IDT[:tp, :tp])
                    nc.scalar.copy(out=hT[:, m, t0:t0 + tp], in_=tps[:, :tp])

            # K^T [C, N] in 5 chunks of 448
            for m in range(3):
                for ci in range(5):
                    n0 = ci * KW
                    kps = ktp.tile([P, KW], f32, tag="kt")
                    for kk in range(3):
                        nc.tensor.matmul(out=kps[:, :],
                                         lhsT=mm(WK[:, kk, m * P:(m + 1) * P]),
                                         rhs=mm(hT[:, kk, n0:n0 + KW]),
                                         start=(kk == 0), stop=(kk == 2))
                    nc.vector.tensor_copy(KT[:, m, n0:n0 + KW], kps[:, :])
                    nc.scalar.copy(out=KTB[:, m, n0:n0 + KW], in_=kps[:, :])

            # V [N, C]
            for bt, (t0, tp) in enumerate(TOKB):
                vps = vpp.tile([P, C], f32, tag="v")
                for kk in range(3):
                    nc.tensor.matmul(out=vps[:tp, :],
                                     lhsT=mm(hT[:, kk, t0:t0 + tp]),
                                     rhs=mm(WV[:, kk, :]),
                                     start=(kk == 0), stop=(kk == 2))
                nc.scalar.copy(out=Vt[0:tp, bt, :], in_=vps[:tp, :])

            # Q^T [C, NLOC] (+ folded-LN bias, scaled by hd^-0.5 on host)
            for m in range(3):
                for (q0, qw) in QCH:
                    qps = qtp.tile([P, 280], f32, tag="qt")
                    for kk in range(3):
                        nc.tensor.matmul(out=qps[:, :qw],
                                         lhsT=mm(WQ[:, kk, m * P:(m + 1) * P]),
                                         rhs=mm(hT[:, kk, q0:q0 + qw]),
                                         start=(kk == 0), stop=(kk == 2))
                    nc.vector.tensor_scalar(out=QT[:, m, q0:q0 + qw],
                                            in0=qps[:, :qw],
                                            scalar1=BQ[:, m:m + 1], scalar2=None,
                                            op0=ALU.add)
                    nc.scalar.activation(out=QTB[:, m, q0:q0 + qw],
                                         in_=qps[:, :qw], func=AF.Identity,
                                         bias=BQ[:, m:m + 1])

        # ---------------- Phase 3: attention ------------------------------
        with ExitStack() as ph:
            o1p = ph.enter_context(tc.tile_pool(name="o1psum", bufs=1, space="PSUM"))
            o2p = ph.enter_context(tc.tile_pool(name="o2psum", bufs=1, space="PSUM"))
            otp = ph.enter_context(tc.tile_pool(name="otpsum", bufs=1, space="PSUM"))
            aup = ph.enter_context(tc.tile_pool(name="attnu", bufs=3))
            autp = ph.enter_context(tc.tile_pool(name="attnut", bufs=3))
            smp = ph.enter_context(tc.tile_pool(name="sums", bufs=4))
            rrp = ph.enter_context(tc.tile_pool(name="rsrow", bufs=2))

            for h in range(H):
                hm, hr = divmod(h, 2)
                QTh = QT[hr * HD:(hr + 1) * HD, hm, :]   # [64, NLOC]
                KTh = KT[hr * HD:(hr + 1) * HD, hm, :]   # [64, N]
                QTBh = QTB[hr * HD:(hr + 1) * HD, hm, :]
                KTBh = KTB[hr * HD:(hr + 1) * HD, hm, :]
                rsrow = rrp.tile([1, NLOC], f32, tag="rsrow")

                # -- orientation 1: attn[q, k] for the HBM write + row sums
                for qi, (q0, qw) in enumerate(QB):
                    attnu = aup.tile([P, N], f32, tag="au")
                    sums = smp.tile([P, 4], f32, tag="sums")
                    for pi, tag in ((0, "o1A"), (1, "o1B")):
                        ps = o1p.tile([P, 1024], f32, tag=tag)
                        for j in range(2):
                            n0 = (pi * 2 + j) * KW
                            nc.tensor.matmul(out=ps[:qw, j * 512:j * 512 + KW],
                                             lhsT=mm(QTh[:, q0:q0 + qw]),
                                             rhs=mm(KTh[:, n0:n0 + KW]),
                                             start=True, stop=True)
                        nc.scalar.activation(
                            out=attnu[0:qw, pi * 2 * KW:(pi * 2 + 2) * KW]
                                .rearrange("p (c n) -> p c n", c=2),
                            in_=ps[0:qw, :].rearrange("p (c n) -> p c n", c=2)[:, :, 0:KW],
                            func=AF.Exp, accum_out=sums[0:qw, pi:pi + 1])
                    ps = o1p.tile([P, 1024], f32, tag="o1A")
                    nc.tensor.matmul(out=ps[:qw, 0:KW],
                                     lhsT=mm(QTh[:, q0:q0 + qw]),
                                     rhs=mm(KTh[:, 4 * KW:5 * KW]),
                                     start=True, stop=True)
                    nc.scalar.activation(out=attnu[0:qw, 4 * KW:5 * KW],
                                         in_=ps[0:qw, 0:KW],
                                         func=AF.Exp, accum_out=sums[0:qw, 2:3])

                    rs = smp.tile([P, 1], f32, tag="rs")
                    nc.vector.reduce_sum(out=rs[:qw, :], in_=sums[0:qw, 0:3],
                                         axis=AX.X)
                    nc.vector.reciprocal(out=rs[:qw, :], in_=rs[:qw, :])
                    nc.vector.tensor_scalar(out=attnu[0:qw, :], in0=attnu[0:qw, :],
                                            scalar1=rs[:qw, :], scalar2=None,
                                            op0=ALU.mult)
                    nc.sync.dma_start(out=dattn[h, q0:q0 + qw, :],
                                      in_=attnu[0:qw, :])

                    tps = o1p.tile([P, 1024], f32, tag="o1A")
                    nc.tensor.transpose(out=tps[0:1, 0:qw], in_=rs[0:qw, 0:1],
                                        identity=IDT[:qw, :qw])
                    nc.scalar.copy(out=rsrow[0:1, q0:q0 + qw], in_=tps[0:1, 0:qw])

                # reciprocal row sums -> broadcast over 64 partitions
                rsbc = rrp.tile([HD, NLOC], f32, tag="rsbc")
                nc.sync.dma_start(out=drs[h:h + 1, :], in_=rsrow[0:1, :])
                nc.sync.dma_start(out=rsbc[:, :],
                                  in_=drs[h:h + 1, :].to_broadcast([HD, NLOC]))

                # -- orientation 2: attnU^T[k, q] and o^T = V^T @ attnU^T
                ot0 = otp.tile([HD, 280], f32, tag="ot0")
                ot1 = otp.tile([HD, 280], f32, tag="ot1")
                for bt, (t0, tp) in enumerate(TOKB):
                    o2ps = o2p.tile([P, 1024], f32, tag="o2")
                    for j, (q0, qw) in enumerate(QCH):
                        nc.tensor.matmul(out=o2ps[:tp, j * 512:j * 512 + qw],
                                         lhsT=KTBh[:, t0:t0 + tp],
                                         rhs=QTBh[:, q0:q0 + qw],
                                         start=True, stop=True)
                    attnut = autp.tile([P, NLOC], bf16, tag="aut")
                    nc.scalar.activation(
                        out=attnut[0:tp, :].rearrange("p (c n) -> p c n", c=2),
                        in_=o2ps[0:tp, :].rearrange("p (c n) -> p c n", c=2)[:, :, 0:280],
                        func=AF.Exp)
                    for j, ot in ((0, ot0), (1, ot1)):
                        nc.tensor.matmul(out=ot[:, :],
                                         lhsT=Vt[0:tp, bt, h * HD:(h + 1) * HD],
                                         rhs=attnut[0:tp, j * 280:(j + 1) * 280],
                                         start=(bt == 0), stop=(bt == len(TOKB) - 1))
                for j, ot in ((0, ot0), (1, ot1)):
                    nc.vector.tensor_mul(
                        out=OT[hr * HD:(hr + 1) * HD, hm, j * 280:(j + 1) * 280],
                        in0=ot[:, :], in1=rsbc[:, j * 280:(j + 1) * 280])

        # ---------------- Phase 4: proj + residual + LN2 + h2^T -----------
        with ExitStack() as ph:
            h2Tp = ph.enter_context(tc.tile_pool(name="h2Tp", bufs=1))
            h2T = h2Tp.tile([P, 3, NLOC], MMDT, tag="h2T")

          # proj/LN2 pools close before the MLP pools open (PSUM budget)
            phA = ph.enter_context(ExitStack())
            pjp = phA.enter_context(tc.tile_pool(name="pjpsum", bufs=2, space="PSUM"))
            t2p = phA.enter_context(tc.tile_pool(name="t2psum", bufs=2, space="PSUM"))
            lnp2 = phA.enter_context(tc.tile_pool(name="ln2", bufs=4))

            for qi, (q0, qw) in enumerate(QB):
                pj = pjp.tile([P, C], f32, tag="pj")
                for kk in range(3):
                    nc.tensor.matmul(out=pj[:qw, :],
                                     lhsT=mm(OT[:, kk, q0:q0 + qw]),
                                     rhs=mm(WP[:, kk, :]),
                                     start=(kk == 0), stop=False)
                nc.tensor.matmul(out=pj[:qw, :], lhsT=mm(ONES[0:1, 0:qw]),
                                 rhs=mm(BP[0:1, :]), start=False, stop=True)
                nc.vector.tensor_add(out=XPL[0:qw, qi, :], in0=XPL[0:qw, qi, :],
                                     in1=pj[:qw, :])
                h2 = lnp2.tile([P, C], f32, tag="h2")
                layernorm(lnp2, XPL[0:qw, qi, :], h2[:qw], qw)
                for m in range(3):
                    tps = t2p.tile([P, P], f32, tag="t2")
                    nc.tensor.transpose(out=tps[:, :qw],
                                        in_=h2[:qw, m * P:(m + 1) * P],
                                        identity=IDT[:qw, :qw])
                    nc.scalar.copy(out=h2T[:, m, q0:q0 + qw], in_=tps[:, :qw])

            phA.close()

            # ------------- Phase 5: MLP -----------------------------------
            with ExitStack() as ph2:
                f1p = ph2.enter_context(tc.tile_pool(name="f1psum", bufs=3,
                                                     space="PSUM"))
                f2p = ph2.enter_context(tc.tile_pool(name="f2psum", bufs=4,
                                                     space="PSUM"))
                gtp = ph2.enter_context(tc.tile_pool(name="gT", bufs=1))
                yp = ph2.enter_context(tc.tile_pool(name="yout", bufs=3))

                gT = gtp.tile([P, 12, NLOC], MMDT, tag="gT")
                for fm in range(12):
                    for (q0, qw) in QCH:
                        fps = f1p.tile([P, 280], f32, tag="f1")
                        for kk in range(3):
                            nc.tensor.matmul(out=fps[:, :qw],
                                             lhsT=mm(WF1[:, kk, fm * P:(fm + 1) * P]),
                                             rhs=mm(h2T[:, kk, q0:q0 + qw]),
                                             start=(kk == 0), stop=(kk == 2))
                        nc.scalar.activation(out=gT[:, fm, q0:q0 + qw],
                                             in_=fps[:, :qw], func=AF.Gelu,
                                             bias=BF1[:, fm:fm + 1])

                for qi, (q0, qw) in enumerate(QB):
                    f2 = f2p.tile([P, C], f32, tag="f2")
                    for kk in range(12):
                        nc.tensor.matmul(out=f2[:qw, :],
                                         lhsT=mm(gT[:, kk, q0:q0 + qw]),
                                         rhs=mm(WF2[:, kk, :]),
                                         start=(kk == 0), stop=False)
                    nc.tensor.matmul(out=f2[:qw, :], lhsT=mm(ONES[0:1, 0:qw]),
                                     rhs=mm(BF2[0:1, :]), start=False, stop=True)
                    yt = yp.tile([P, C], f32, tag="y")
                    nc.vector.tensor_add(out=yt[:qw, :], in0=XPL[0:qw, qi, :],
                                         in1=f2[:qw, :])
                    nc.sync.dma_start(out=dy[q0:q0 + qw, :], in_=yt[:qw, :])

    nc.finalize()
    return nc


def _get_program():
    global _PROG
    if _PROG is None:
        _PROG = build_program()
    return _PROG


def tf32_round(a):
    """Round fp32 -> tf32 (10-bit mantissa, RNE). fp32r == tf32 on trn2."""
    if not USE_F32R:
        return np.asarray(a, np.float32)
    u = np.asarray(a, np.float32).view(np.uint32).astype(np.uint64)
    u = (u + 0x0FFF + ((u >> 13) & 1)) & np.uint64(0xFFFFE000)
    return u.astype(np.uint32).view(np.float32)


def host_weights(inputs):
    """Fold LN gains/biases into the weights; pre-transpose for the PE."""
    f64 = np.float64
    w_qkv = np.asarray(inputs["w_qkv"], f64)
    ln1_g = np.asarray(inputs["ln1_g"], f64)
    ln1_b = np.asarray(inputs["ln1_b"], f64)
    ln2_g = np.asarray(inputs["ln2_g"], f64)
    ln2_b = np.asarray(inputs["ln2_b"], f64)
    w_proj = np.asarray(inputs["w_proj"], f64)
    b_proj = np.asarray(inputs["b_proj"], f64)
    w_fc1 = np.asarray(inputs["w_fc1"], f64)
    b_fc1 = np.asarray(inputs["b_fc1"], f64)
    w_fc2 = np.asarray(inputs["w_fc2"], f64)
    b_fc2 = np.asarray(inputs["b_fc2"], f64)

    scale = HD ** -0.5
    wq = w_qkv[0:C] * ln1_g[None, :] * scale
    bq = (w_qkv[0:C] @ ln1_b) * scale
    wk = w_qkv[C:2 * C] * ln1_g[None, :]
    # k-bias shifts every logit in a softmax row by a constant -> no-op.
    wv = w_qkv[2 * C:3 * C] * ln1_g[None, :]
    bv = w_qkv[2 * C:3 * C] @ ln1_b
    # v-bias adds a constant vector to every o row (attn rows sum to 1):
    # fold it through the projection into the projection bias.
    bp = b_proj + w_proj @ bv
    wf1 = w_fc1 * ln2_g[None, :]
    bf1 = b_fc1 + w_fc1 @ ln2_b

    def c(a):
        return np.ascontiguousarray(a)

    def cr(a):
        return np.ascontiguousarray(tf32_round(a.astype(np.float32)))

    return {
        "wqT": cr(wq.T),
        "wkT": cr(wk.T),
        "wvT": cr(wv.T),
        "wpT": cr(w_proj.T),
        "wf1T": cr(wf1.T),
        "wf2T": cr(w_fc2.T),
        "bq": c(bq.astype(np.float32).reshape(3, P).T),
        "bf1": c(bf1.astype(np.float32).reshape(12, P).T),
        "bp": cr(bp.reshape(1, C)),
        "bf2": cr(np.asarray(b_fc2, np.float64).reshape(1, C)),
    }


def kernel(**inputs):
    global LAST_RESULT
    from concourse import bass_utils

    x = np.asarray(inputs["x"], np.float32)                 # [2, N, C]
    pos = np.asarray(inputs["encoder_pos"], np.float32)
    rep = N // pos.shape[1]
    if rep != 1:
        pos = np.repeat(pos, rep, axis=1)
    pos = pos[0]                                            # [N, C]
    base = host_weights(inputs)

    in_maps = []
    for core in range(NCORES):
        b = core // 4
        qs = (core % 4) * NLOC
        in_maps.append({
            **base,
            "x": np.ascontiguousarray(np.roll(x[b], -qs, axis=0)),
            "pos": np.ascontiguousarray(np.roll(pos, -qs, axis=0)),
        })

    nc = _get_program()
    kw = {}
    tdir = os.environ.get("BASS_KERNEL_TRACE_DIR")
    if tdir:
        kw["tmpdir"] = tdir
    res = bass_utils.run_bass_kernel_spmd(nc, in_maps,
                                          core_ids=list(range(NCORES)), **kw)
    LAST_RESULT = res

    attn = np.empty((2, H, N, N), np.float32)
    y = np.empty((2, N, C), np.float32)
    for core in range(NCORES):
        b = core // 4
        qs = (core % 4) * NLOC
        attn[b, :, qs:qs + NLOC, :] = np.roll(res.results[core]["attn_out"],
                                              qs, axis=-1)
        y[b, qs:qs + NLOC, :] = res.results[core]["y_out"]
    return (y, attn)
